# revision 12
# baseline (speedup 1.0000x reference)
"""Trainium2 Bass kernel for nn_CSNNet (conv1d -> maxpool -> 25-step LIF SNN -> fc -> LIF).

Strategy (v3): FEATURE-parallel across 8 cores; 4-engine pipeline per core.
---------------------------------------------------------------------------
Each core holds ALL 256 batches but 1/8 of the pooled feature positions
(8 channels x 512 positions = 4096 features = 32 contraction chunks of 128).
Host sums the per-core partial fc products g_t at the end.

Math: with m_t the layer-1 membrane AFTER the step-t update (m_0 = cur1), the
snntorch Leaky recurrence on the device's NEGATED NORMALIZED membrane
mh_t = -m_t/thr is
    mh_{t+1} = beta*mh_t + CUR + (mh_t < -1),   CUR = -cur1/thr = mh_0
and W@spk_t is recovered on the host from g_t = wt.T @ mh_t via
    W@spk_t ~ g_{t+1} - beta*g_t - g_0.

Per-step engine split of the 8192-wide membrane tile (free = chunk*256 + b):
  PE    : u = beta*mh + CUR for chunks 0..23 via identity-matmul pairs into
          PSUM (12+12 N=512 matmuls), plus the 32 N=256 g-matmuls.
  DVE   : mh' = (mh < -1) + u as ONE scalar_tensor_tensor per 1536-col
          sub-chunk (in0 SBUF, in1 PSUM -> the GPSIMD-shared SBUF port stays
          free), 4 sub-chunks/step.
  GPSIMD: chunks 24..31 (2048 cols) with the stock two-op recurrence,
          SBUF-only, running concurrently with the DVE's PSUM-source ops.
  ACT   : conv affine tails + PSUM->SBUF drains of the g accumulators.

Conv (pad=1, k=3, maxpool2): taps materialized host-side per partition
(xw[p, m*1024 + q*256 + b] = x_pad[b, 1024*core + 256q + 2p + m]); PE
computes e1 = r01*a0 + a1 and o1 = r01*a1 + a2 into PSUM with scaled
identities; DVE finishes e2/o2 (+ max) and ACT applies the final affine.

Layout (per core)
-----------------
  partition p + chunk ch <-> channel c = ch//4, position jl = 128*(ch%4)+p
  mh/cur [128, 8192]  free index = ch*256 + b
  xw     [128, 4096]  tap m slice = [:, 1024m : 1024m+1024], inner (q, b)
  wt     [128, 64]    wt[p, 2ch+o] = fc_w[o, c*4096 + 512*core + 128*(ch%4)+p]
  idn    [128, 1280]  10 identities: beta*I, I, r01_c*I for c in 0..7
  g psum: per t, 4 col-groups x [2, 256] rows 32g..32g+2; 4 t-slots in 2
          banks, ACT-drained every 2 steps into gsb [128, 26*256].
"""

import numpy as np

BETA = 0.9
NUM_STEPS = 25
B_FULL, L, C = 256, 8192, 8
NCORES = 8
NP = 128                        # partitions
B = B_FULL                      # batches per core (all of them)
JL = 512                        # pooled positions per core
NCH = 32                        # contraction chunks of 128 features
NT = NUM_STEPS + 1              # 26 membrane states m_0..m_25
FREE = NCH * B                  # 8192 free columns
N_PI = 24                       # chunks handled by the PE-u + DVE psum path
N_PSI = 4                       # chunks handled by DVE's own two-op path
N_GP = NCH - N_PI - N_PSI       # chunks handled by GPSIMD (four-op path)
PI = N_PI * B                   # 6144
PSI = N_PSI * B                 # 1024
SUB = 1536                      # DVE sub-chunk width (3 PSUM banks)
NSUB = PI // SUB                # 4 sub-chunks per step
DPS = NSUB + 1                  # dve_pi increments per step (subs + psi)

_PROG_CACHE = {}

# test-harness knobs (defaults are what the grader sees: no profiling)
PROFILE = False
TRACE_DIR = None
LAST = {}


def _conv_scalars(conv_w, conv_b, thr1):
    """Per-channel immediates for the Horner-style conv chains.

    E = w0*A(-1) + w1*A(0) + w2*A(1) + b   (even output of the pool pair)
    O = w0*A(0)  + w1*A(1) + w2*A(2) + b   (odd)
    computed as e2 = (A(-1)*(w0/w1) + A(0))*(w1/w2) + A(1)  (x w2, +b folded
    into the final affine), and max(E,O) = w2*max(e2,o2)+b for w2>0,
    w2*min(e2,o2)+b for w2<0.  Output is CUR = -(max(E,O)+b)/thr.
    """
    out = []
    for c in range(C):
        w0, w1, w2 = (float(conv_w[c, 0, d]) for d in range(3))
        b = float(conv_b[c])
        assert abs(w1) > 1e-6 and abs(w2) > 1e-6, "degenerate conv weights"
        r01 = np.float32(w0 / w1)
        r12 = np.float32(w1 / w2)
        use_max = w2 > 0
        sA = np.float32(-w2 / thr1)
        sB = np.float32(-b / thr1)
        out.append((float(r01), float(r12), use_max, float(sA), float(sB)))
    return out


def _build_nc(conv_w, conv_b, thr1):
    """Build the single-core Bass program (SPMD-identical on all 8 cores)."""
    import concourse.bass as bass
    import concourse.mybir as mybir
    from concourse.alu_op_type import AluOpType as alu
    from contextlib import ExitStack

    f32 = mybir.dt.float32
    nc = bass.Bass()
    csc = _conv_scalars(conv_w, conv_b, thr1)

    xw = nc.dram_tensor("xw", [NP, 4096], f32, kind="ExternalInput")
    wt = nc.dram_tensor("wt", [NP, 2 * NCH], f32, kind="ExternalInput")
    idn = nc.dram_tensor("idn", [NP, 128 * (2 + C)], f32, kind="ExternalInput")
    g_out = nc.dram_tensor("g_out", [8, NT * B], f32, kind="ExternalOutput")

    with ExitStack() as es:
        dma_in = es.enter_context(nc.semaphore("dma_in"))
        pe_cv = es.enter_context(nc.semaphore("pe_cv"))    # conv e1/o1 psum ready
        cv_dve = es.enter_context(nc.semaphore("cv_dve"))  # conv e2/o2 done (psum free)
        conv_sem = es.enter_context(nc.semaphore("conv_sem"))  # ACT ts per channel
        pe_u = es.enter_context(nc.semaphore("pe_u"))      # u sub-chunks ready
        dve_pi = es.enter_context(nc.semaphore("dve_pi"))  # pi sub-chunk steps done
        gp_sem = es.enter_context(nc.semaphore("gp_sem"))  # gpsimd region steps done
        pe_g = es.enter_context(nc.semaphore("pe_g"))      # g-groups accumulated
        scl_g = es.enter_context(nc.semaphore("scl_g"))    # g banks drained
        out_sem = es.enter_context(nc.semaphore("out_sem"))

        xw_sb = es.enter_context(nc.sbuf_tensor("xw_sb", [NP, 4096], f32))
        wt_sb = es.enter_context(nc.sbuf_tensor("wt_sb", [NP, 2 * NCH], f32))
        idn_sb = es.enter_context(nc.sbuf_tensor("idn_sb", [NP, 128 * (2 + C)], f32))
        cur = es.enter_context(nc.sbuf_tensor("cur", [NP, FREE], f32))
        mA = es.enter_context(nc.sbuf_tensor("mA", [NP, FREE], f32))
        mB = es.enter_context(nc.sbuf_tensor("mB", [NP, FREE], f32))
        gU = es.enter_context(nc.sbuf_tensor("gU", [NP, N_GP * B], f32))
        gV = es.enter_context(nc.sbuf_tensor("gV", [NP, N_GP * B], f32))
        gS = es.enter_context(nc.sbuf_tensor("gS", [NP, N_GP * B], f32))
        uPsi = es.enter_context(nc.sbuf_tensor("uPsi", [NP, PSI], f32))
        ce2 = es.enter_context(nc.sbuf_tensor("ce2", [NP, 1024], f32))
        co2 = es.enter_context(nc.sbuf_tensor("co2", [NP, 1024], f32))
        mx0 = es.enter_context(nc.sbuf_tensor("mx0", [NP, 1024], f32))
        mx1 = es.enter_context(nc.sbuf_tensor("mx1", [NP, 1024], f32))
        gsb = es.enter_context(nc.sbuf_tensor("gsb", [NP, NT * B], f32))
        u0 = es.enter_context(nc.psum_tensor("u0", [NP, SUB], f32))
        u1 = es.enter_context(nc.psum_tensor("u1", [NP, SUB], f32))
        g0 = es.enter_context(nc.psum_tensor("g0", [NP, 512], f32))
        g1 = es.enter_context(nc.psum_tensor("g1", [NP, 512], f32))
        block = es.enter_context(nc.Block())

        bI = idn_sb[:, 0:128]            # beta * I
        I_ = idn_sb[:, 128:256]          # I
        rI = [idn_sb[:, 128 * (2 + c) : 128 * (3 + c)] for c in range(C)]
        a_m = [xw_sb[:, 1024 * m : 1024 * (m + 1)] for m in range(4)]
        ubuf = [u0, u1]
        gps = [g0, g1]

        def mbuf(k):        # buffer holding membrane state mh_k
            if k == 0:
                return cur
            return mA if (k % 2 == 1) else mB

        @block.sync
        def _(sync):
            sync.dma_start(out=idn_sb[:], in_=idn[:]).then_inc(dma_in, 16)
            sync.dma_start(out=wt_sb[:], in_=wt[:]).then_inc(dma_in, 16)
            sync.dma_start(out=xw_sb[:], in_=xw[:]).then_inc(dma_in, 16)
            sync.wait_ge(scl_g, NT // 2)
            for j in range(4):
                sync.dma_start(
                    out=g_out[2 * j : 2 * j + 2, :],
                    in_=gsb[32 * j : 32 * j + 2, :],
                ).then_inc(out_sem, 16)
            sync.wait_ge(out_sem, 64)

        @block.scalar
        def _(scalar):
            # conv: final affine per channel, trailing the DVE max
            for c in range(C):
                _, _, _, sA, sB = csc[c]
                scalar.wait_ge(cv_dve, 2 * c + 2)  # mx{c%2} written
                scalar.activation(
                    out=cur[:, 1024 * c : 1024 * (c + 1)],
                    in_=(mx0 if c % 2 == 0 else mx1)[:],
                    func=mybir.ActivationFunctionType.Copy,
                    bias=float(sB), scale=float(sA),
                ).then_inc(conv_sem)
            # g drains: bank k%2 holds steps (2k, 2k+1)
            for k in range(NT // 2):
                scalar.wait_ge(pe_g, 2 * k + 2)
                ins = None
                for j in range(4):
                    ins = scalar.copy(
                        out=gsb[32 * j : 32 * j + 2, 2 * k * B : (2 * k + 2) * B],
                        in_=gps[k % 2][32 * j : 32 * j + 2, :],
                    )
                ins.then_inc(scl_g)

        @block.tensor
        def _(tensor):
            tensor.wait_ge(dma_in, 48)
            # ---- conv: e1 = r01*a0 + a1 -> u0[0:1024]; o1 = r01*a1 + a2 -> u1[0:1024]
            for c in range(C):
                r01 = rI[c]
                if c >= 1:
                    tensor.wait_ge(cv_dve, 2 * c - 1)  # DVE done reading psum of c-1
                ins = None
                for piece in range(2):
                    s = slice(512 * piece, 512 * (piece + 1))
                    tensor.matmul(u0[:, s], r01, a_m[0][:, s], start=True, stop=False)
                    tensor.matmul(u1[:, s], r01, a_m[1][:, s], start=True, stop=False)
                for piece in range(2):
                    s = slice(512 * piece, 512 * (piece + 1))
                    tensor.matmul(u0[:, s], I_, a_m[1][:, s], start=False, stop=True)
                    ins = tensor.matmul(
                        u1[:, s], I_, a_m[2][:, s], start=False, stop=True
                    )
                ins.then_inc(pe_cv)  # pe_cv = c+1
            # ---- recurrence ----
            for t in range(NUM_STEPS + 1):
                # u = beta*mh_t + CUR for chunks 0..23 (sub-chunks of 1536),
                # paired 2 at a time to amortize the big identity loads.
                # (u for step t is consumed by the DVE producing mh_{t+1};
                #  the last u pass is t=NUM_STEPS-1.)
                if t < NUM_STEPS:
                    for jp in range(NSUB // 2):       # sub-chunk pairs (0,1), (2,3)
                        subs = [2 * jp, 2 * jp + 1]
                        for j in subs:
                            # buffer guard: ubuf[j%2] last read by DVE sub-chunk
                            # j-2 of this step / j+2 of the previous step
                            w = DPS * t + j - 2 if j < 2 else DPS * t + j - 1
                            if t == 0 and jp == 0:
                                tensor.wait_ge(conv_sem, C)  # mh_0 = cur ready
                            elif w > 0:
                                tensor.wait_ge(dve_pi, w)
                        ins = None
                        for j in subs:
                            for piece in range(SUB // 512):
                                s = slice(j * SUB + 512 * piece,
                                          j * SUB + 512 * (piece + 1))
                                ps = slice(512 * piece, 512 * (piece + 1))
                                tensor.matmul(
                                    ubuf[j % 2][:, ps], bI, mbuf(t)[:, s],
                                    start=True, stop=False,
                                )
                        for j in subs:
                            for piece in range(SUB // 512):
                                s = slice(j * SUB + 512 * piece,
                                          j * SUB + 512 * (piece + 1))
                                ps = slice(512 * piece, 512 * (piece + 1))
                                ins = tensor.matmul(
                                    ubuf[j % 2][:, ps], I_, cur[:, s],
                                    start=False, stop=True,
                                )
                        ins.then_inc(pe_u, 2)  # pe_u = NSUB*t + j + 1
                # g_t = wt.T @ mh_t, accumulated over 32 chunks into 4
                # column-group tiles of the step's psum slot.
                if t >= 1:
                    tensor.wait_ge(dve_pi, DPS * t)    # pi+psi part of mh_t done
                    tensor.wait_ge(gp_sem, t)          # gpsimd part of mh_t done
                else:
                    tensor.wait_ge(conv_sem, C)
                if t >= 4:
                    tensor.wait_ge(scl_g, (t - 4) // 2 + 1)  # slot drained
                ps = gps[(t % 4) // 2]
                col = (t % 2) * B
                mm = None
                for ch in range(NCH):
                    j = ch % 4
                    mm = tensor.matmul(
                        ps[32 * j : 32 * j + 2, col : col + B],
                        wt_sb[:, 2 * ch : 2 * ch + 2],
                        mbuf(t)[:, B * ch : B * (ch + 1)],
                        start=(ch < 4),
                        stop=(ch >= NCH - 4),
                        skip_group_check=True,
                        tile_position=(0, 32 * j),
                    )
                mm.then_inc(pe_g)  # pe_g = t+1

        @block.vector
        def _(vector):
            vector.wait_ge(dma_in, 48)
            # ---- conv: e2 = r12*e1 + a2 ; o2 = r12*o1 + a3 ; mx = max/min
            for c in range(C):
                r01, r12, use_max, sA, sB = csc[c]
                vector.wait_ge(pe_cv, c + 1)
                vector.scalar_tensor_tensor(
                    out=ce2[:], in0=u0[:, 0:1024], scalar=r12, in1=a_m[2][:],
                    op0=alu.mult, op1=alu.add,
                )
                vector.scalar_tensor_tensor(
                    out=co2[:], in0=u1[:, 0:1024], scalar=r12, in1=a_m[3][:],
                    op0=alu.mult, op1=alu.add,
                ).then_inc(cv_dve)  # psum of channel c free
                vector.tensor_tensor(
                    out=(mx0 if c % 2 == 0 else mx1)[:], in0=ce2[:], in1=co2[:],
                    op=(alu.max if use_max else alu.min),
                ).then_inc(cv_dve)  # cv_dve = 2c+2 : mx ready for ACT
            # ---- recurrence: mh_{t+1} = (mh_t < -1) + u, one stt per sub-chunk
            # (in1 from PSUM keeps the GPSIMD-shared SBUF port free), plus the
            # psi region with the classic two-op form.
            ps_sl = slice(PI, PI + PSI)
            for t in range(NUM_STEPS):
                vector.wait_ge(pe_g, t)  # g_{t-1} read out of mbuf(t+1)
                for j in range(NSUB):
                    vector.wait_ge(pe_u, NSUB * t + j + 1)
                    s = slice(j * SUB, (j + 1) * SUB)
                    vector.scalar_tensor_tensor(
                        out=mbuf(t + 1)[:, s], in0=mbuf(t)[:, s], scalar=-1.0,
                        in1=ubuf[j % 2][:], op0=alu.is_lt, op1=alu.add,
                    ).then_inc(dve_pi)  # dve_pi = DPS*t + j + 1
                vector.scalar_tensor_tensor(
                    out=uPsi[:], in0=mbuf(t)[:, ps_sl], scalar=BETA,
                    in1=cur[:, ps_sl], op0=alu.mult, op1=alu.add,
                )
                vector.scalar_tensor_tensor(
                    out=mbuf(t + 1)[:, ps_sl], in0=mbuf(t)[:, ps_sl],
                    scalar=-1.0, in1=uPsi[:], op0=alu.is_lt, op1=alu.add,
                ).then_inc(dve_pi)  # dve_pi = DPS*t + NSUB + 1

        @block.gpsimd
        def _(gpsimd):
            gpsimd.wait_ge(dma_in, 48)
            gpsimd.wait_ge(conv_sem, C)
            gs = slice(PI + PSI, FREE)
            for t in range(NUM_STEPS):
                if t >= 1:
                    gpsimd.wait_ge(pe_g, t)  # g_{t-1} read out of mbuf(t+1)
                # Pool has no fused stt / tensor-compare: four stock ops.
                gpsimd.tensor_scalar(
                    out=gS[:], in0=mbuf(t)[:, gs], scalar1=-1.0, scalar2=None,
                    op0=alu.is_lt,
                )
                gpsimd.tensor_scalar(
                    out=gV[:], in0=mbuf(t)[:, gs], scalar1=BETA, scalar2=None,
                    op0=alu.mult,
                )
                gpsimd.tensor_tensor(
                    out=gU[:], in0=gV[:], in1=cur[:, gs], op=alu.add,
                )
                gpsimd.tensor_tensor(
                    out=mbuf(t + 1)[:, gs], in0=gU[:], in1=gS[:], op=alu.add,
                ).then_inc(gp_sem)  # gp_sem = t+1 : gp part of mh_{t+1} ready

    return nc


def _prep_inputs(x, fc_w):
    """Host-side layout prep: conv tap windows + fc weight permute + identities."""
    x = np.ascontiguousarray(np.asarray(x, np.float32).reshape(B_FULL, L))
    x_pad = np.zeros((B_FULL, L + 3), np.float32)
    x_pad[:, 1 : L + 1] = x

    fc_w = np.asarray(fc_w, np.float32)
    # wt[p, 2*(4c+q)+o] = fc_w[o, c*4096 + 512*i + 128*q + p]
    fcv = fc_w.reshape(2, C, NCORES, 4, NP)          # [o, c, i, q, p]
    wts = []
    xws = []
    s = x_pad.strides
    for i in range(NCORES):
        arr = fcv[:, :, i]                           # [o, c, q, p]
        wt = np.ascontiguousarray(arr.transpose(3, 1, 2, 0)).reshape(NP, 2 * NCH)
        wts.append(wt)
        # xw[p, 1024m + 256q + b] = x_pad[b, 1024i + 256q + 2p + m]
        win = np.lib.stride_tricks.as_strided(
            x_pad[:, 1024 * i :],
            shape=(B_FULL, 4, NP, 4),                # [b, q, p, m]
            strides=(s[0], 256 * s[1], 2 * s[1], s[1]),
        )
        xws.append(
            np.ascontiguousarray(win.transpose(2, 3, 1, 0)).reshape(NP, 4096)
        )
    return xws, wts


def _prep_idn(csc):
    idn = np.zeros((NP, 128 * (2 + C)), np.float32)
    eye = np.eye(NP, dtype=np.float32)
    idn[:, 0:128] = np.float32(BETA) * eye
    idn[:, 128:256] = eye
    for c in range(C):
        idn[:, 128 * (2 + c) : 128 * (3 + c)] = np.float32(csc[c][0]) * eye
    return idn


def kernel(x, conv_w, conv_b, fc_w, fc_b, thr1, thr_out):
    from concourse.bass_utils import run_bass_kernel_spmd

    conv_w = np.asarray(conv_w, np.float32)
    conv_b = np.asarray(conv_b, np.float32)
    fc_b = np.asarray(fc_b, np.float64)
    thr1_f = float(np.asarray(thr1))
    thr_out_f = float(np.asarray(thr_out))

    key = (conv_w.tobytes(), conv_b.tobytes(), thr1_f)
    nc = _PROG_CACHE.get(key)
    if nc is None:
        nc = _build_nc(conv_w, conv_b, thr1_f)
        _PROG_CACHE[key] = nc

    xws, wts = _prep_inputs(x, fc_w)
    idn = _prep_idn(_conv_scalars(conv_w, conv_b, thr1_f))
    in_maps = [{"xw": xws[i], "wt": wts[i], "idn": idn} for i in range(NCORES)]
    res = run_bass_kernel_spmd(
        nc, in_maps, list(range(NCORES)),
        trace=PROFILE, tmpdir=TRACE_DIR,
    )
    LAST["exec_time_ns"] = res.exec_time_ns
    LAST["trace"] = res.instructions_and_trace

    # host-side: sum partial g over cores + col groups, recover cur_out, then
    # the tiny output-layer recurrence in numpy.
    gtot = np.zeros((2, NT, B), np.float64)
    for i in range(NCORES):
        g = np.asarray(res.results[i]["g_out"], np.float64)  # [8, 26*256]
        gtot += g.reshape(4, 2, NT, B).sum(axis=0)
    # g_t = -(W@m_t)/thr, so W@spk_t = g_{t+1} - beta*g_t - g_0 (thr cancels)
    wr = gtot[:, 1:] - BETA * gtot[:, :NUM_STEPS] - gtot[:, :1]
    cur_out = wr.transpose(1, 2, 0) + fc_b[None, None, :]

    mem = np.zeros((B_FULL, 2), np.float64)
    spk_rec = np.empty((NUM_STEPS, B_FULL, 2), np.float32)
    mem_rec = np.empty((NUM_STEPS, B_FULL, 2), np.float32)
    for t in range(NUM_STEPS):
        reset = (mem > thr_out_f).astype(np.float64)
        mem = BETA * mem + cur_out[t] - reset * thr_out_f
        spk_rec[t] = (mem > thr_out_f).astype(np.float32)
        mem_rec[t] = mem.astype(np.float32)
    return spk_rec, mem_rec


# revision 18
# speedup vs baseline: 1.0417x; 1.0417x over previous
"""Trainium2 Bass kernel for nn_CSNNet (conv1d -> maxpool -> 25-step LIF SNN -> fc -> LIF).

Strategy (v4): FEATURE-parallel across 8 cores; PE+DVE pipeline per core.
---------------------------------------------------------------------------
Each core holds ALL 256 batches but 1/8 of the pooled feature positions
(8 channels x 512 positions = 4096 features = 32 contraction chunks of 128).
Host sums the per-core partial fc products g_t at the end.

Math: with m_t the layer-1 membrane AFTER the step-t update (m_0 = cur1), the
snntorch Leaky recurrence on the device's NEGATED NORMALIZED membrane
mh_t = -m_t/thr is
    mh_{t+1} = beta*mh_t + CUR + (mh_t < -1),   CUR = -cur1/thr = mh_0
and W@spk_t is recovered on the host from g_t = wt.T @ mh_t via
    W@spk_t ~ g_{t+1} - beta*g_t - g_0.

Per-step engine split of the 8192-wide membrane tile (free = chunk*256 + b):
  PE    : u = beta*mh + CUR for chunks 0..27 via identity-matmul pairs into
          PSUM (fp32 matmuls are LOW/HIGH two-pass, ~1ns/col), plus the 32
          N=256 g-matmuls in float32r (single-pass) with 4-way column tiling.
  DVE   : mh' = (mh < -1) + u as ONE scalar_tensor_tensor per 1024-col
          sub-chunk (in0 SBUF, in1 PSUM), 7 sub-chunks/step through 3
          rotating PSUM buffers, plus the last 4 chunks with the classic
          two-op form (psi region).
  ACT   : conv affine tails + PSUM->SBUF drains of the g accumulators.

Conv (pad=1, k=3, maxpool2): taps materialized host-side per partition
(xw[p, m*1024 + q*256 + b] = x_pad[b, 1024*core + 256q + 2p + m]); PE
computes e1 = r01*a0 + a1 and o1 = r01*a1 + a2 into PSUM with scaled
identities; DVE finishes e2/o2 (+ max) and ACT applies the final affine.

Layout (per core)
-----------------
  partition p + chunk ch <-> channel c = ch//4, position jl = 128*(ch%4)+p
  mh/cur [128, 8192]  free index = ch*256 + b
  xw     [128, 4096]  tap m slice = [:, 1024m : 1024m+1024], inner (q, b)
  wt     [128, 64]    wt[p, 2ch+o] = fc_w[o, c*4096 + 512*core + 128*(ch%4)+p]
  idn    [128, 1280]  10 identities: beta*I, I, r01_c*I for c in 0..7
  g psum: per t, 4 col-groups x [2, 256] rows 32g..32g+2; 4 t-slots in 2
          banks, ACT-drained every 2 steps into gsb [128, 26*256].
"""

import numpy as np

BETA = 0.9
NUM_STEPS = 25
B_FULL, L, C = 256, 8192, 8
NCORES = 8
NP = 128                        # partitions
B = B_FULL                      # batches per core (all of them)
JL = 512                        # pooled positions per core
NCH = 32                        # contraction chunks of 128 features
NT = NUM_STEPS + 1              # 26 membrane states m_0..m_25
FREE = NCH * B                  # 8192 free columns
N_PI = 28                       # chunks handled by the PE-u + DVE psum path
N_PSI = NCH - N_PI              # chunks handled by DVE's own two-op path
PI = N_PI * B                   # 7168
PSI = N_PSI * B                 # 1024
SUB = 1024                      # DVE sub-chunk width (2 PSUM banks)
NSUB = PI // SUB                # 7 sub-chunks per step
NBUF = 3                        # rotating u PSUM buffers
DPS = NSUB + 1                  # dve_pi increments per step (subs + psi)

_PROG_CACHE = {}

# test-harness knobs (defaults are what the grader sees: no profiling)
PROFILE = False
TRACE_DIR = None
LAST = {}


def _conv_scalars(conv_w, conv_b, thr1):
    """Per-channel immediates for the Horner-style conv chains.

    E = w0*A(-1) + w1*A(0) + w2*A(1) + b   (even output of the pool pair)
    O = w0*A(0)  + w1*A(1) + w2*A(2) + b   (odd)
    computed as e2 = (A(-1)*(w0/w1) + A(0))*(w1/w2) + A(1)  (x w2, +b folded
    into the final affine), and max(E,O) = w2*max(e2,o2)+b for w2>0,
    w2*min(e2,o2)+b for w2<0.  Output is CUR = -(max(E,O)+b)/thr.
    """
    out = []
    for c in range(C):
        w0, w1, w2 = (float(conv_w[c, 0, d]) for d in range(3))
        b = float(conv_b[c])
        assert abs(w1) > 1e-6 and abs(w2) > 1e-6, "degenerate conv weights"
        r01 = np.float32(w0 / w1)
        r12 = np.float32(w1 / w2)
        use_max = w2 > 0
        sA = np.float32(-w2 / thr1)
        sB = np.float32(-b / thr1)
        out.append((float(r01), float(r12), use_max, float(sA), float(sB)))
    return out


def _build_nc(conv_w, conv_b, thr1, g_f32r=False):
    """Build the single-core Bass program (SPMD-identical on all 8 cores)."""
    import concourse.bass as bass
    import concourse.mybir as mybir
    from concourse.alu_op_type import AluOpType as alu
    from contextlib import ExitStack

    f32 = mybir.dt.float32
    f32r = mybir.dt.float32r
    nc = bass.Bass()
    csc = _conv_scalars(conv_w, conv_b, thr1)

    xw = nc.dram_tensor("xw", [NP, 4096], f32, kind="ExternalInput")
    wt = nc.dram_tensor("wt", [NP, 2 * NCH], f32, kind="ExternalInput")
    idn = nc.dram_tensor("idn", [NP, 128 * (2 + C)], f32, kind="ExternalInput")
    g_out = nc.dram_tensor("g_out", [8, NT * B], f32, kind="ExternalOutput")

    with ExitStack() as es:
        dma_in = es.enter_context(nc.semaphore("dma_in"))
        pe_cv = es.enter_context(nc.semaphore("pe_cv"))    # conv e1/o1 psum ready
        cv_dve = es.enter_context(nc.semaphore("cv_dve"))  # conv e2/o2 done (psum free)
        conv_sem = es.enter_context(nc.semaphore("conv_sem"))  # ACT ts per channel
        pe_u = es.enter_context(nc.semaphore("pe_u"))      # u sub-chunks ready
        dve_pi = es.enter_context(nc.semaphore("dve_pi"))  # dve per-step progress
        pe_g = es.enter_context(nc.semaphore("pe_g"))      # g-groups accumulated
        scl_g = es.enter_context(nc.semaphore("scl_g"))    # g banks drained
        out_sem = es.enter_context(nc.semaphore("out_sem"))

        xw_sb = es.enter_context(nc.sbuf_tensor("xw_sb", [NP, 4096], f32))
        wt_sb = es.enter_context(nc.sbuf_tensor("wt_sb", [NP, 2 * NCH], f32))
        idn_sb = es.enter_context(nc.sbuf_tensor("idn_sb", [NP, 128 * (2 + C)], f32))
        cur = es.enter_context(nc.sbuf_tensor("cur", [NP, FREE], f32))
        mA = es.enter_context(nc.sbuf_tensor("mA", [NP, FREE], f32))
        mB = es.enter_context(nc.sbuf_tensor("mB", [NP, FREE], f32))
        uPsi = es.enter_context(nc.sbuf_tensor("uPsi", [NP, PSI], f32))
        ce2 = es.enter_context(nc.sbuf_tensor("ce2", [NP, 1024], f32))
        co2 = es.enter_context(nc.sbuf_tensor("co2", [NP, 1024], f32))
        mx0 = es.enter_context(nc.sbuf_tensor("mx0", [NP, 1024], f32))
        mx1 = es.enter_context(nc.sbuf_tensor("mx1", [NP, 1024], f32))
        gsb = es.enter_context(nc.sbuf_tensor("gsb", [NP, NT * B], f32))
        ubufs = [
            es.enter_context(nc.psum_tensor(f"u{i}", [NP, SUB], f32))
            for i in range(NBUF)
        ]
        g0 = es.enter_context(nc.psum_tensor("g0", [NP, 512], f32))
        g1 = es.enter_context(nc.psum_tensor("g1", [NP, 512], f32))
        block = es.enter_context(nc.Block())

        bI = idn_sb[:, 0:128]            # beta * I
        I_ = idn_sb[:, 128:256]          # I
        rI = [idn_sb[:, 128 * (2 + c) : 128 * (3 + c)] for c in range(C)]
        a_m = [xw_sb[:, 1024 * m : 1024 * (m + 1)] for m in range(4)]
        gps = [g0, g1]
        if g_f32r:
            wt_g = wt_sb[:].bitcast(f32r)
        else:
            wt_g = wt_sb[:]

        def mbuf(k):        # buffer holding membrane state mh_k
            if k == 0:
                return cur
            return mA if (k % 2 == 1) else mB

        def mbuf_g(k):      # g-matmul rhs view (optionally float32r)
            return mbuf(k)[:].bitcast(f32r) if g_f32r else mbuf(k)[:]

        @block.sync
        def _(sync):
            sync.dma_start(out=idn_sb[:], in_=idn[:]).then_inc(dma_in, 16)
            sync.dma_start(out=wt_sb[:], in_=wt[:]).then_inc(dma_in, 16)
            sync.dma_start(out=xw_sb[:], in_=xw[:]).then_inc(dma_in, 16)
            sync.wait_ge(scl_g, NT // 2)
            for j in range(4):
                sync.dma_start(
                    out=g_out[2 * j : 2 * j + 2, :],
                    in_=gsb[32 * j : 32 * j + 2, :],
                ).then_inc(out_sem, 16)
            sync.wait_ge(out_sem, 64)

        @block.scalar
        def _(scalar):
            # conv: final affine per channel, trailing the DVE max
            for c in range(C):
                _, _, _, sA, sB = csc[c]
                scalar.wait_ge(cv_dve, 2 * c + 2)  # mx{c%2} written
                scalar.activation(
                    out=cur[:, 1024 * c : 1024 * (c + 1)],
                    in_=(mx0 if c % 2 == 0 else mx1)[:],
                    func=mybir.ActivationFunctionType.Copy,
                    bias=float(sB), scale=float(sA),
                ).then_inc(conv_sem)
            # g drains: bank k%2 holds steps (2k, 2k+1)
            for k in range(NT // 2):
                scalar.wait_ge(pe_g, 2 * k + 2)
                ins = None
                for j in range(4):
                    ins = scalar.copy(
                        out=gsb[32 * j : 32 * j + 2, 2 * k * B : (2 * k + 2) * B],
                        in_=gps[k % 2][32 * j : 32 * j + 2, :],
                    )
                ins.then_inc(scl_g)

        @block.tensor
        def _(tensor):
            tensor.wait_ge(dma_in, 48)
            # ---- conv: e1 = r01*a0 + a1 -> u0; o1 = r01*a1 + a2 -> u1
            for c in range(C):
                r01 = rI[c]
                if c >= 1:
                    tensor.wait_ge(cv_dve, 2 * c - 1)  # DVE done reading psum of c-1
                ins = None
                for piece in range(2):
                    s = slice(512 * piece, 512 * (piece + 1))
                    tensor.matmul(ubufs[0][:, s], r01, a_m[0][:, s],
                                  start=True, stop=False)
                    tensor.matmul(ubufs[1][:, s], r01, a_m[1][:, s],
                                  start=True, stop=False)
                for piece in range(2):
                    s = slice(512 * piece, 512 * (piece + 1))
                    tensor.matmul(ubufs[0][:, s], I_, a_m[1][:, s],
                                  start=False, stop=True)
                    ins = tensor.matmul(
                        ubufs[1][:, s], I_, a_m[2][:, s], start=False, stop=True
                    )
                ins.then_inc(pe_cv)  # pe_cv = c+1
            # ---- recurrence ----
            for t in range(NUM_STEPS + 1):
                # u = beta*mh_t + CUR for chunks 0..27, sub-chunks of 1024
                # through 3 rotating psum buffers (consumed by the DVE's
                # fused compare+add producing mh_{t+1}).
                if t < NUM_STEPS:
                    for j in range(NSUB):
                        k = NSUB * t + j
                        if t == 0 and j == 0:
                            tensor.wait_ge(conv_sem, C)  # mh_0 = cur ready
                        if k >= NBUF:
                            t2, j2 = divmod(k - NBUF, NSUB)
                            tensor.wait_ge(dve_pi, DPS * t2 + j2 + 1)
                        ub = ubufs[k % NBUF]
                        s0 = j * SUB
                        ins = None
                        for piece in range(SUB // 512):
                            s = slice(s0 + 512 * piece, s0 + 512 * (piece + 1))
                            ps = slice(512 * piece, 512 * (piece + 1))
                            tensor.matmul(ub[:, ps], bI, mbuf(t)[:, s],
                                          start=True, stop=False)
                        for piece in range(SUB // 512):
                            s = slice(s0 + 512 * piece, s0 + 512 * (piece + 1))
                            ps = slice(512 * piece, 512 * (piece + 1))
                            ins = tensor.matmul(ub[:, ps], I_, cur[:, s],
                                                start=False, stop=True)
                        ins.then_inc(pe_u)  # pe_u = NSUB*t + j + 1
                # g_t = wt.T @ mh_t, accumulated over 32 chunks into 4
                # column-group tiles of the step's psum slot.
                if t >= 1:
                    tensor.wait_ge(dve_pi, DPS * t)    # all of mh_t written
                else:
                    tensor.wait_ge(conv_sem, C)
                if t >= 4:
                    tensor.wait_ge(scl_g, (t - 4) // 2 + 1)  # slot drained
                ps = gps[(t % 4) // 2]
                col = (t % 2) * B
                rhs = mbuf_g(t)
                mm = None
                for ch in range(NCH):
                    j = ch % 4
                    mm = tensor.matmul(
                        ps[32 * j : 32 * j + 2, col : col + B],
                        wt_g[:, 2 * ch : 2 * ch + 2],
                        rhs[:, B * ch : B * (ch + 1)],
                        start=(ch < 4),
                        stop=(ch >= NCH - 4),
                        skip_group_check=True,
                        tile_position=(0, 32 * j),
                    )
                mm.then_inc(pe_g)  # pe_g = t+1

        @block.vector
        def _(vector):
            vector.wait_ge(dma_in, 48)
            # ---- conv: e2 = r12*e1 + a2 ; o2 = r12*o1 + a3 ; mx = max/min
            for c in range(C):
                r01, r12, use_max, sA, sB = csc[c]
                vector.wait_ge(pe_cv, c + 1)
                vector.scalar_tensor_tensor(
                    out=ce2[:], in0=ubufs[0][:], scalar=r12, in1=a_m[2][:],
                    op0=alu.mult, op1=alu.add,
                )
                vector.scalar_tensor_tensor(
                    out=co2[:], in0=ubufs[1][:], scalar=r12, in1=a_m[3][:],
                    op0=alu.mult, op1=alu.add,
                ).then_inc(cv_dve)  # psum of channel c free
                vector.tensor_tensor(
                    out=(mx0 if c % 2 == 0 else mx1)[:], in0=ce2[:], in1=co2[:],
                    op=(alu.max if use_max else alu.min),
                ).then_inc(cv_dve)  # cv_dve = 2c+2 : mx ready for ACT
            # ---- recurrence: mh_{t+1} = (mh_t < -1) + u, one stt per
            # sub-chunk (in1 from PSUM), plus the psi region's two-op form.
            vector.wait_ge(conv_sem, C)
            ps_sl = slice(PI, PI + PSI)
            for t in range(NUM_STEPS):
                vector.wait_ge(pe_g, t)  # g_{t-1} read out of mbuf(t+1)
                for j in range(NSUB):
                    k = NSUB * t + j
                    vector.wait_ge(pe_u, k + 1)
                    s = slice(j * SUB, (j + 1) * SUB)
                    vector.scalar_tensor_tensor(
                        out=mbuf(t + 1)[:, s], in0=mbuf(t)[:, s], scalar=-1.0,
                        in1=ubufs[k % NBUF][:], op0=alu.is_lt, op1=alu.add,
                    ).then_inc(dve_pi)  # dve_pi = DPS*t + j + 1
                vector.scalar_tensor_tensor(
                    out=uPsi[:], in0=mbuf(t)[:, ps_sl], scalar=BETA,
                    in1=cur[:, ps_sl], op0=alu.mult, op1=alu.add,
                )
                vector.scalar_tensor_tensor(
                    out=mbuf(t + 1)[:, ps_sl], in0=mbuf(t)[:, ps_sl],
                    scalar=-1.0, in1=uPsi[:], op0=alu.is_lt, op1=alu.add,
                ).then_inc(dve_pi)  # dve_pi = DPS*t + NSUB + 1

    return nc


def _prep_inputs(x, fc_w):
    """Host-side layout prep: conv tap windows + fc weight permute."""
    x = np.ascontiguousarray(np.asarray(x, np.float32).reshape(B_FULL, L))
    x_pad = np.zeros((B_FULL, L + 3), np.float32)
    x_pad[:, 1 : L + 1] = x

    fc_w = np.asarray(fc_w, np.float32)
    # wt[p, 2*(4c+q)+o] = fc_w[o, c*4096 + 512*i + 128*q + p]
    fcv = fc_w.reshape(2, C, NCORES, 4, NP)          # [o, c, i, q, p]
    wts = []
    xws = []
    s = x_pad.strides
    for i in range(NCORES):
        arr = fcv[:, :, i]                           # [o, c, q, p]
        wt = np.ascontiguousarray(arr.transpose(3, 1, 2, 0)).reshape(NP, 2 * NCH)
        wts.append(wt)
        # xw[p, 1024m + 256q + b] = x_pad[b, 1024i + 256q + 2p + m]
        win = np.lib.stride_tricks.as_strided(
            x_pad[:, 1024 * i :],
            shape=(B_FULL, 4, NP, 4),                # [b, q, p, m]
            strides=(s[0], 256 * s[1], 2 * s[1], s[1]),
        )
        xws.append(
            np.ascontiguousarray(win.transpose(2, 3, 1, 0)).reshape(NP, 4096)
        )
    return xws, wts


def _prep_idn(csc):
    idn = np.zeros((NP, 128 * (2 + C)), np.float32)
    eye = np.eye(NP, dtype=np.float32)
    idn[:, 0:128] = np.float32(BETA) * eye
    idn[:, 128:256] = eye
    for c in range(C):
        idn[:, 128 * (2 + c) : 128 * (3 + c)] = np.float32(csc[c][0]) * eye
    return idn


def kernel(x, conv_w, conv_b, fc_w, fc_b, thr1, thr_out):
    from concourse.bass_utils import run_bass_kernel_spmd

    conv_w = np.asarray(conv_w, np.float32)
    conv_b = np.asarray(conv_b, np.float32)
    fc_b = np.asarray(fc_b, np.float64)
    thr1_f = float(np.asarray(thr1))
    thr_out_f = float(np.asarray(thr_out))

    key = (conv_w.tobytes(), conv_b.tobytes(), thr1_f)
    nc = _PROG_CACHE.get(key)
    if nc is None:
        nc = _build_nc(conv_w, conv_b, thr1_f)
        _PROG_CACHE[key] = nc

    xws, wts = _prep_inputs(x, fc_w)
    idn = _prep_idn(_conv_scalars(conv_w, conv_b, thr1_f))
    in_maps = [{"xw": xws[i], "wt": wts[i], "idn": idn} for i in range(NCORES)]
    res = run_bass_kernel_spmd(
        nc, in_maps, list(range(NCORES)),
        trace=PROFILE, tmpdir=TRACE_DIR,
    )
    LAST["exec_time_ns"] = res.exec_time_ns
    LAST["trace"] = res.instructions_and_trace

    # host-side: sum partial g over cores + col groups, recover cur_out, then
    # the tiny output-layer recurrence in numpy.
    gtot = np.zeros((2, NT, B), np.float64)
    for i in range(NCORES):
        g = np.asarray(res.results[i]["g_out"], np.float64)  # [8, 26*256]
        gtot += g.reshape(4, 2, NT, B).sum(axis=0)
    # g_t = -(W@m_t)/thr, so W@spk_t = g_{t+1} - beta*g_t - g_0 (thr cancels)
    wr = gtot[:, 1:] - BETA * gtot[:, :NUM_STEPS] - gtot[:, :1]
    cur_out = wr.transpose(1, 2, 0) + fc_b[None, None, :]

    mem = np.zeros((B_FULL, 2), np.float64)
    spk_rec = np.empty((NUM_STEPS, B_FULL, 2), np.float32)
    mem_rec = np.empty((NUM_STEPS, B_FULL, 2), np.float32)
    for t in range(NUM_STEPS):
        reset = (mem > thr_out_f).astype(np.float64)
        mem = BETA * mem + cur_out[t] - reset * thr_out_f
        spk_rec[t] = (mem > thr_out_f).astype(np.float32)
        mem_rec[t] = mem.astype(np.float32)
    return spk_rec, mem_rec


# revision 19
# speedup vs baseline: 1.5598x; 1.4974x over previous
"""Trainium2 Bass kernel for nn_CSNNet (conv1d -> maxpool -> 25-step LIF SNN -> fc -> LIF).

Strategy (v4): FEATURE-parallel across 8 cores; PE+DVE pipeline per core.
---------------------------------------------------------------------------
Each core holds ALL 256 batches but 1/8 of the pooled feature positions
(8 channels x 512 positions = 4096 features = 32 contraction chunks of 128).
Host sums the per-core partial fc products g_t at the end.

Math: with m_t the layer-1 membrane AFTER the step-t update (m_0 = cur1), the
snntorch Leaky recurrence on the device's NEGATED NORMALIZED membrane
mh_t = -m_t/thr is
    mh_{t+1} = beta*mh_t + CUR + (mh_t < -1),   CUR = -cur1/thr = mh_0
and W@spk_t is recovered on the host from g_t = wt.T @ mh_t via
    W@spk_t ~ g_{t+1} - beta*g_t - g_0.

Per-step engine split of the 8192-wide membrane tile (free = chunk*256 + b):
  PE    : u = beta*mh + CUR for chunks 0..27 via identity-matmul pairs into
          PSUM (fp32 matmuls are LOW/HIGH two-pass, ~1ns/col), plus the 32
          N=256 g-matmuls in float32r (single-pass) with 4-way column tiling.
  DVE   : mh' = (mh < -1) + u as ONE scalar_tensor_tensor per 1024-col
          sub-chunk (in0 SBUF, in1 PSUM), 7 sub-chunks/step through 3
          rotating PSUM buffers, plus the last 4 chunks with the classic
          two-op form (psi region).
  ACT   : conv affine tails + PSUM->SBUF drains of the g accumulators.

Conv (pad=1, k=3, maxpool2): taps materialized host-side per partition
(xw[p, m*1024 + q*256 + b] = x_pad[b, 1024*core + 256q + 2p + m]); PE
computes e1 = r01*a0 + a1 and o1 = r01*a1 + a2 into PSUM with scaled
identities; DVE finishes e2/o2 (+ max) and ACT applies the final affine.

Layout (per core)
-----------------
  partition p + chunk ch <-> channel c = ch//4, position jl = 128*(ch%4)+p
  mh/cur [128, 8192]  free index = ch*256 + b
  xw     [128, 4096]  tap m slice = [:, 1024m : 1024m+1024], inner (q, b)
  wt     [128, 64]    wt[p, 2ch+o] = fc_w[o, c*4096 + 512*core + 128*(ch%4)+p]
  idn    [128, 1280]  10 identities: beta*I, I, r01_c*I for c in 0..7
  g psum: per t, 4 col-groups x [2, 256] rows 32g..32g+2; 4 t-slots in 2
          banks, ACT-drained every 2 steps into gsb [128, 26*256].
"""

import numpy as np

BETA = 0.9
NUM_STEPS = 25
B_FULL, L, C = 256, 8192, 8
NCORES = 8
NP = 128                        # partitions
B = B_FULL                      # batches per core (all of them)
JL = 512                        # pooled positions per core
NCH = 32                        # contraction chunks of 128 features
NT = NUM_STEPS + 1              # 26 membrane states m_0..m_25
FREE = NCH * B                  # 8192 free columns

_PROG_CACHE = {}

# test-harness knobs (defaults are what the grader sees: no profiling)
PROFILE = False
TRACE_DIR = None
LAST = {}


def _conv_scalars(conv_w, conv_b, thr1):
    """Per-channel immediates for the Horner-style conv chains.

    E = w0*A(-1) + w1*A(0) + w2*A(1) + b   (even output of the pool pair)
    O = w0*A(0)  + w1*A(1) + w2*A(2) + b   (odd)
    computed as e2 = (A(-1)*(w0/w1) + A(0))*(w1/w2) + A(1)  (x w2, +b folded
    into the final affine), and max(E,O) = w2*max(e2,o2)+b for w2>0,
    w2*min(e2,o2)+b for w2<0.  Output is CUR = -(max(E,O)+b)/thr.
    """
    out = []
    for c in range(C):
        w0, w1, w2 = (float(conv_w[c, 0, d]) for d in range(3))
        b = float(conv_b[c])
        assert abs(w1) > 1e-6 and abs(w2) > 1e-6, "degenerate conv weights"
        r01 = np.float32(w0 / w1)
        r12 = np.float32(w1 / w2)
        use_max = w2 > 0
        sA = np.float32(-w2 / thr1)
        sB = np.float32(-b / thr1)
        out.append((float(r01), float(r12), use_max, float(sA), float(sB)))
    return out


def _build_nc(conv_w, conv_b, thr1, g_f32r=False):
    """Build the single-core Bass program (SPMD-identical on all 8 cores)."""
    import concourse.bass as bass
    import concourse.mybir as mybir
    from concourse.alu_op_type import AluOpType as alu
    from contextlib import ExitStack

    f32 = mybir.dt.float32
    f32r = mybir.dt.float32r
    nc = bass.Bass()
    csc = _conv_scalars(conv_w, conv_b, thr1)

    xw = nc.dram_tensor("xw", [NP, 4096], f32, kind="ExternalInput")
    wt = nc.dram_tensor("wt", [NP, 2 * NCH], f32, kind="ExternalInput")
    idn = nc.dram_tensor("idn", [NP, 128 * (2 + C)], f32, kind="ExternalInput")
    g_out = nc.dram_tensor("g_out", [8, NT * B], f32, kind="ExternalOutput")

    with ExitStack() as es:
        dma_in = es.enter_context(nc.semaphore("dma_in"))
        pe_cv = es.enter_context(nc.semaphore("pe_cv"))    # conv e1/o1 psum ready
        cv_dve = es.enter_context(nc.semaphore("cv_dve"))  # conv e2/o2 done (psum free)
        conv_sem = es.enter_context(nc.semaphore("conv_sem"))  # ACT ts per channel
        dve_pi = es.enter_context(nc.semaphore("dve_pi"))  # dve steps done
        h25 = es.enter_context(nc.semaphore("h25"))        # first half of mh_25
        pe_g = es.enter_context(nc.semaphore("pe_g"))      # g-groups accumulated
        scl_g = es.enter_context(nc.semaphore("scl_g"))    # g banks drained
        out_sem = es.enter_context(nc.semaphore("out_sem"))

        xw_sb = es.enter_context(nc.sbuf_tensor("xw_sb", [NP, 4096], f32))
        wt_sb = es.enter_context(nc.sbuf_tensor("wt_sb", [NP, 2 * NCH], f32))
        idn_sb = es.enter_context(nc.sbuf_tensor("idn_sb", [NP, 128 * (2 + C)], f32))
        # pads stagger the big tensors' SBUF base offsets so the two read
        # streams + write stream of each recurrence op land in different
        # bank phases (two-SBUF-source DVE ops otherwise measure ~22% slow)
        cur = es.enter_context(nc.sbuf_tensor("cur", [NP, FREE], f32))
        es.enter_context(nc.sbuf_tensor("pad0", [NP, 8], f32))
        mA = es.enter_context(nc.sbuf_tensor("mA", [NP, FREE], f32))
        es.enter_context(nc.sbuf_tensor("pad1", [NP, 16], f32))
        mB = es.enter_context(nc.sbuf_tensor("mB", [NP, FREE], f32))
        es.enter_context(nc.sbuf_tensor("pad2", [NP, 24], f32))
        uT = es.enter_context(nc.sbuf_tensor("uT", [NP, FREE], f32))
        ce2 = es.enter_context(nc.sbuf_tensor("ce2", [NP, 1024], f32))
        co2 = es.enter_context(nc.sbuf_tensor("co2", [NP, 1024], f32))
        mx0 = es.enter_context(nc.sbuf_tensor("mx0", [NP, 1024], f32))
        mx1 = es.enter_context(nc.sbuf_tensor("mx1", [NP, 1024], f32))
        gsb = es.enter_context(nc.sbuf_tensor("gsb", [NP, NT * B], f32))
        ubufs = [
            es.enter_context(nc.psum_tensor(f"u{i}", [NP, 1024], f32))
            for i in range(2)
        ]
        g0 = es.enter_context(nc.psum_tensor("g0", [NP, 512], f32))
        g1 = es.enter_context(nc.psum_tensor("g1", [NP, 512], f32))
        block = es.enter_context(nc.Block())

        bI = idn_sb[:, 0:128]            # beta * I
        I_ = idn_sb[:, 128:256]          # I
        rI = [idn_sb[:, 128 * (2 + c) : 128 * (3 + c)] for c in range(C)]
        a_m = [xw_sb[:, 1024 * m : 1024 * (m + 1)] for m in range(4)]
        gps = [g0, g1]
        if g_f32r:
            wt_g = wt_sb[:].bitcast(f32r)
        else:
            wt_g = wt_sb[:]

        def mbuf(k):        # buffer holding membrane state mh_k
            if k == 0:
                return cur
            return mA if (k % 2 == 1) else mB

        def mbuf_g(k):      # g-matmul rhs view (optionally float32r)
            return mbuf(k)[:].bitcast(f32r) if g_f32r else mbuf(k)[:]

        @block.sync
        def _(sync):
            sync.dma_start(out=idn_sb[:], in_=idn[:]).then_inc(dma_in, 16)
            sync.dma_start(out=wt_sb[:], in_=wt[:]).then_inc(dma_in, 16)
            sync.dma_start(out=xw_sb[:], in_=xw[:]).then_inc(dma_in, 16)
            sync.wait_ge(scl_g, NT // 2)
            for j in range(4):
                sync.dma_start(
                    out=g_out[2 * j : 2 * j + 2, :],
                    in_=gsb[32 * j : 32 * j + 2, :],
                ).then_inc(out_sem, 16)
            sync.wait_ge(out_sem, 64)

        @block.scalar
        def _(scalar):
            # conv: final affine per channel, trailing the DVE max
            for c in range(C):
                _, _, _, sA, sB = csc[c]
                scalar.wait_ge(cv_dve, 2 * c + 2)  # mx{c%2} written
                scalar.activation(
                    out=cur[:, 1024 * c : 1024 * (c + 1)],
                    in_=(mx0 if c % 2 == 0 else mx1)[:],
                    func=mybir.ActivationFunctionType.Copy,
                    bias=float(sB), scale=float(sA),
                ).then_inc(conv_sem)
            # g drains: bank k%2 holds steps (2k, 2k+1)
            for k in range(NT // 2):
                scalar.wait_ge(pe_g, 2 * k + 2)
                ins = None
                for j in range(4):
                    ins = scalar.copy(
                        out=gsb[32 * j : 32 * j + 2, 2 * k * B : (2 * k + 2) * B],
                        in_=gps[k % 2][32 * j : 32 * j + 2, :],
                    )
                ins.then_inc(scl_g)

        @block.tensor
        def _(tensor):
            tensor.wait_ge(dma_in, 48)
            # ---- conv: e1 = r01*a0 + a1 -> u0; o1 = r01*a1 + a2 -> u1
            for c in range(C):
                r01 = rI[c]
                if c >= 1:
                    tensor.wait_ge(cv_dve, 2 * c - 1)  # DVE done reading psum of c-1
                ins = None
                for piece in range(2):
                    s = slice(512 * piece, 512 * (piece + 1))
                    tensor.matmul(ubufs[0][:, s], r01, a_m[0][:, s],
                                  start=True, stop=False)
                    tensor.matmul(ubufs[1][:, s], r01, a_m[1][:, s],
                                  start=True, stop=False)
                for piece in range(2):
                    s = slice(512 * piece, 512 * (piece + 1))
                    tensor.matmul(ubufs[0][:, s], I_, a_m[1][:, s],
                                  start=False, stop=True)
                    ins = tensor.matmul(
                        ubufs[1][:, s], I_, a_m[2][:, s], start=False, stop=True
                    )
                ins.then_inc(pe_cv)  # pe_cv = c+1
            # ---- recurrence: only the 26 g-matmul groups
            for t in range(NUM_STEPS + 1):
                if t == 0:
                    tensor.wait_ge(conv_sem, C)        # mh_0 = cur ready
                elif t == NUM_STEPS:
                    tensor.wait_ge(h25, 1)             # first half of mh_25
                else:
                    tensor.wait_ge(dve_pi, t)          # mh_t written
                if t >= 4:
                    tensor.wait_ge(scl_g, (t - 4) // 2 + 1)  # slot drained
                ps = gps[(t % 4) // 2]
                col = (t % 2) * B
                rhs = mbuf_g(t)
                mm = None
                for ch in range(NCH):
                    if t == NUM_STEPS and ch == NCH // 2:
                        tensor.wait_ge(dve_pi, NUM_STEPS)  # second half ready
                    j = ch % 4
                    mm = tensor.matmul(
                        ps[32 * j : 32 * j + 2, col : col + B],
                        wt_g[:, 2 * ch : 2 * ch + 2],
                        rhs[:, B * ch : B * (ch + 1)],
                        start=(ch < 4),
                        stop=(ch >= NCH - 4),
                        skip_group_check=True,
                        tile_position=(0, 32 * j),
                    )
                mm.then_inc(pe_g)  # pe_g = t+1

        @block.vector
        def _(vector):
            vector.wait_ge(dma_in, 48)
            # ---- conv: e2 = r12*e1 + a2 ; o2 = r12*o1 + a3 ; mx = max/min
            for c in range(C):
                r01, r12, use_max, sA, sB = csc[c]
                vector.wait_ge(pe_cv, c + 1)
                vector.scalar_tensor_tensor(
                    out=ce2[:], in0=ubufs[0][:], scalar=r12, in1=a_m[2][:],
                    op0=alu.mult, op1=alu.add,
                )
                vector.scalar_tensor_tensor(
                    out=co2[:], in0=ubufs[1][:], scalar=r12, in1=a_m[3][:],
                    op0=alu.mult, op1=alu.add,
                ).then_inc(cv_dve)  # psum of channel c free
                vector.tensor_tensor(
                    out=(mx0 if c % 2 == 0 else mx1)[:], in0=ce2[:], in1=co2[:],
                    op=(alu.max if use_max else alu.min),
                ).then_inc(cv_dve)  # cv_dve = 2c+2 : mx ready for ACT
            # ---- recurrence: whole-tile u = beta*mh + CUR then
            # mh' = (mh < -1) + u; last step in halves so the final g-matmul
            # chain overlaps the second half.
            vector.wait_ge(conv_sem, C)
            for t in range(NUM_STEPS):
                vector.scalar_tensor_tensor(
                    out=uT[:], in0=mbuf(t)[:], scalar=BETA, in1=cur[:],
                    op0=alu.mult, op1=alu.add,
                )
                vector.wait_ge(pe_g, t)  # g_{t-1} read out of mbuf(t+1)
                if t < NUM_STEPS - 1:
                    vector.scalar_tensor_tensor(
                        out=mbuf(t + 1)[:], in0=mbuf(t)[:], scalar=-1.0,
                        in1=uT[:], op0=alu.is_lt, op1=alu.add,
                    ).then_inc(dve_pi)  # dve_pi = t+1
                else:
                    vector.scalar_tensor_tensor(
                        out=mbuf(t + 1)[:, 0:4096], in0=mbuf(t)[:, 0:4096],
                        scalar=-1.0, in1=uT[:, 0:4096],
                        op0=alu.is_lt, op1=alu.add,
                    ).then_inc(h25)
                    vector.scalar_tensor_tensor(
                        out=mbuf(t + 1)[:, 4096:8192],
                        in0=mbuf(t)[:, 4096:8192], scalar=-1.0,
                        in1=uT[:, 4096:8192], op0=alu.is_lt, op1=alu.add,
                    ).then_inc(dve_pi)

    return nc


def _prep_inputs(x, fc_w):
    """Host-side layout prep: conv tap windows + fc weight permute."""
    x = np.ascontiguousarray(np.asarray(x, np.float32).reshape(B_FULL, L))
    x_pad = np.zeros((B_FULL, L + 3), np.float32)
    x_pad[:, 1 : L + 1] = x

    fc_w = np.asarray(fc_w, np.float32)
    # wt[p, 2*(4c+q)+o] = fc_w[o, c*4096 + 512*i + 128*q + p]
    fcv = fc_w.reshape(2, C, NCORES, 4, NP)          # [o, c, i, q, p]
    wts = []
    xws = []
    s = x_pad.strides
    for i in range(NCORES):
        arr = fcv[:, :, i]                           # [o, c, q, p]
        wt = np.ascontiguousarray(arr.transpose(3, 1, 2, 0)).reshape(NP, 2 * NCH)
        wts.append(wt)
        # xw[p, 1024m + 256q + b] = x_pad[b, 1024i + 256q + 2p + m]
        win = np.lib.stride_tricks.as_strided(
            x_pad[:, 1024 * i :],
            shape=(B_FULL, 4, NP, 4),                # [b, q, p, m]
            strides=(s[0], 256 * s[1], 2 * s[1], s[1]),
        )
        xws.append(
            np.ascontiguousarray(win.transpose(2, 3, 1, 0)).reshape(NP, 4096)
        )
    return xws, wts


def _prep_idn(csc):
    idn = np.zeros((NP, 128 * (2 + C)), np.float32)
    eye = np.eye(NP, dtype=np.float32)
    idn[:, 0:128] = np.float32(BETA) * eye
    idn[:, 128:256] = eye
    for c in range(C):
        idn[:, 128 * (2 + c) : 128 * (3 + c)] = np.float32(csc[c][0]) * eye
    return idn


def kernel(x, conv_w, conv_b, fc_w, fc_b, thr1, thr_out):
    from concourse.bass_utils import run_bass_kernel_spmd

    conv_w = np.asarray(conv_w, np.float32)
    conv_b = np.asarray(conv_b, np.float32)
    fc_b = np.asarray(fc_b, np.float64)
    thr1_f = float(np.asarray(thr1))
    thr_out_f = float(np.asarray(thr_out))

    key = (conv_w.tobytes(), conv_b.tobytes(), thr1_f)
    nc = _PROG_CACHE.get(key)
    if nc is None:
        nc = _build_nc(conv_w, conv_b, thr1_f)
        _PROG_CACHE[key] = nc

    xws, wts = _prep_inputs(x, fc_w)
    idn = _prep_idn(_conv_scalars(conv_w, conv_b, thr1_f))
    in_maps = [{"xw": xws[i], "wt": wts[i], "idn": idn} for i in range(NCORES)]
    res = run_bass_kernel_spmd(
        nc, in_maps, list(range(NCORES)),
        trace=PROFILE, tmpdir=TRACE_DIR,
    )
    LAST["exec_time_ns"] = res.exec_time_ns
    LAST["trace"] = res.instructions_and_trace

    # host-side: sum partial g over cores + col groups, recover cur_out, then
    # the tiny output-layer recurrence in numpy.
    gtot = np.zeros((2, NT, B), np.float64)
    for i in range(NCORES):
        g = np.asarray(res.results[i]["g_out"], np.float64)  # [8, 26*256]
        gtot += g.reshape(4, 2, NT, B).sum(axis=0)
    # g_t = -(W@m_t)/thr, so W@spk_t = g_{t+1} - beta*g_t - g_0 (thr cancels)
    wr = gtot[:, 1:] - BETA * gtot[:, :NUM_STEPS] - gtot[:, :1]
    cur_out = wr.transpose(1, 2, 0) + fc_b[None, None, :]

    mem = np.zeros((B_FULL, 2), np.float64)
    spk_rec = np.empty((NUM_STEPS, B_FULL, 2), np.float32)
    mem_rec = np.empty((NUM_STEPS, B_FULL, 2), np.float32)
    for t in range(NUM_STEPS):
        reset = (mem > thr_out_f).astype(np.float64)
        mem = BETA * mem + cur_out[t] - reset * thr_out_f
        spk_rec[t] = (mem > thr_out_f).astype(np.float32)
        mem_rec[t] = mem.astype(np.float32)
    return spk_rec, mem_rec


# revision 21
# speedup vs baseline: 1.9805x; 1.2697x over previous
"""Trainium2 Bass kernel for nn_CSNNet (conv1d -> maxpool -> 25-step LIF SNN -> fc -> LIF).

Strategy (v7): FEATURE-parallel across 8 cores.
-----------------------------------------------
Each core holds ALL 256 batches but 1/8 of the pooled feature positions
(8 channels x 512 positions = 4096 features = 32 contraction chunks of 128).
Host sums the per-core partial fc products g_t at the end.

Math: with m_t the layer-1 membrane AFTER the step-t update (m_0 = cur1), the
snntorch Leaky recurrence on the device's NEGATED NORMALIZED membrane
mh_t = -m_t/thr is
    mh_{t+1} = beta*mh_t + CUR + (mh_t < -1),   CUR = -cur1/thr = mh_0
and W@spk_t is recovered on the host from g_t = wt.T @ mh_t via
    W@spk_t ~ g_{t+1} - beta*g_t - g_0.

Engine schedule:
  DVE    : conv chains + the recurrence. Per step, pass A
           u = beta*mh + CUR writes INTO PSUM chunks, pass B
           mh' = (mh < -1) + u reads u back from PSUM: a PSUM in1 avoids the
           ~25% second-SBUF-port penalty measured on two-SBUF-source ops.
  PE     : the 26 g_t = wt.T @ mh_t accumulations (32 N=256 fp32 matmuls per
           step, 4-way column-tiled; ~4us/step, hidden under the DVE).
  ACT    : conv per-channel affine tails + PSUM->SBUF drains of g.

Conv (pad=1, k=3, maxpool2): tap-separated windows materialized host-side
(xw[p, m*1024 + q*256 + b] = x_pad[b, 1024*core + 256q + 2p + m]) make all
chain operands contiguous; per channel the DVE runs the Horner chains
e2 = (a0*r01 + a1)*r12 + a2, o2 = (a1*r01 + a2)*r12 + a3, mx = max/min, and
ACT applies CUR = mx*sA + sB.

Layout (per core)
-----------------
  partition p + chunk ch <-> channel c = ch//4, position jl = 128*(ch%4)+p
  mh/cur [128, 8192]  free index = ch*256 + b
  xw     [128, 4096]  tap m slice = [:, 1024m : 1024m+1024], inner (q, b)
  wt     [128, 64]    wt[p, 2ch+o] = fc_w[o, c*4096 + 512*core + 128*(ch%4)+p]
  uP     [128, 3584]  PSUM staging for pass A (7 banks); g: 1 bank, 2 t-slots,
                      ACT-drained every 2 steps into gsb [128, 26*256].
"""

import numpy as np

BETA = 0.9
NUM_STEPS = 25
B_FULL, L, C = 256, 8192, 8
NCORES = 8
NP = 128                        # partitions
B = B_FULL                      # batches per core (all of them)
NCH = 32                        # contraction chunks of 128 features
NT = NUM_STEPS + 1              # 26 membrane states m_0..m_25
FREE = NCH * B                  # 8192 free columns
UW = 3584                       # PSUM pass-A staging width (7 banks)

_PROG_CACHE = {}

# test-harness knobs (defaults are what the grader sees: no profiling)
PROFILE = False
TRACE_DIR = None
LAST = {}


def _conv_scalars(conv_w, conv_b, thr1):
    """Per-channel immediates for the Horner-style conv chains.

    E = w0*A(-1) + w1*A(0) + w2*A(1) + b   (even output of the pool pair)
    O = w0*A(0)  + w1*A(1) + w2*A(2) + b   (odd)
    computed as e2 = (A(-1)*(w0/w1) + A(0))*(w1/w2) + A(1)  (x w2, +b folded
    into the final affine), and max(E,O) = w2*max(e2,o2)+b for w2>0,
    w2*min(e2,o2)+b for w2<0.  Output is CUR = -(max(E,O)+b)/thr.
    """
    out = []
    for c in range(C):
        w0, w1, w2 = (float(conv_w[c, 0, d]) for d in range(3))
        b = float(conv_b[c])
        assert abs(w1) > 1e-6 and abs(w2) > 1e-6, "degenerate conv weights"
        r01 = np.float32(w0 / w1)
        r12 = np.float32(w1 / w2)
        use_max = w2 > 0
        sA = np.float32(-w2 / thr1)
        sB = np.float32(-b / thr1)
        out.append((float(r01), float(r12), use_max, float(sA), float(sB)))
    return out


def _build_nc(conv_w, conv_b, thr1):
    """Build the single-core Bass program (SPMD-identical on all 8 cores)."""
    import concourse.bass as bass
    import concourse.mybir as mybir
    from concourse.alu_op_type import AluOpType as alu
    from contextlib import ExitStack

    f32 = mybir.dt.float32
    nc = bass.Bass()
    csc = _conv_scalars(conv_w, conv_b, thr1)

    xw = nc.dram_tensor("xw", [NP, 4096], f32, kind="ExternalInput")
    wt = nc.dram_tensor("wt", [NP, 2 * NCH], f32, kind="ExternalInput")
    g_out = nc.dram_tensor("g_out", [8, NT * B], f32, kind="ExternalOutput")

    with ExitStack() as es:
        dma_in = es.enter_context(nc.semaphore("dma_in"))
        cv_dve = es.enter_context(nc.semaphore("cv_dve"))  # conv mx per channel
        conv_sem = es.enter_context(nc.semaphore("conv_sem"))  # ACT affine per ch
        dve_pi = es.enter_context(nc.semaphore("dve_pi"))  # dve steps done
        h25 = es.enter_context(nc.semaphore("h25"))        # chunks 0..27 of mh_25
        pe_g = es.enter_context(nc.semaphore("pe_g"))      # g-groups accumulated
        scl_g = es.enter_context(nc.semaphore("scl_g"))    # g slots drained
        out_sem = es.enter_context(nc.semaphore("out_sem"))

        xw_sb = es.enter_context(nc.sbuf_tensor("xw_sb", [NP, 4096], f32))
        wt_sb = es.enter_context(nc.sbuf_tensor("wt_sb", [NP, 2 * NCH], f32))
        cur = es.enter_context(nc.sbuf_tensor("cur", [NP, FREE], f32))
        mA = es.enter_context(nc.sbuf_tensor("mA", [NP, FREE], f32))
        mB = es.enter_context(nc.sbuf_tensor("mB", [NP, FREE], f32))
        cvE = es.enter_context(nc.sbuf_tensor("cvE", [NP, 1024], f32))
        cvO = es.enter_context(nc.sbuf_tensor("cvO", [NP, 1024], f32))
        ce2 = es.enter_context(nc.sbuf_tensor("ce2", [NP, 1024], f32))
        co2 = es.enter_context(nc.sbuf_tensor("co2", [NP, 1024], f32))
        mx0 = es.enter_context(nc.sbuf_tensor("mx0", [NP, 1024], f32))
        mx1 = es.enter_context(nc.sbuf_tensor("mx1", [NP, 1024], f32))
        gsb = es.enter_context(nc.sbuf_tensor("gsb", [NP, NT * B], f32))
        uP = es.enter_context(nc.psum_tensor("uP", [NP, UW], f32))
        g0 = es.enter_context(nc.psum_tensor("g0", [NP, 512], f32))
        block = es.enter_context(nc.Block())

        a_m = [xw_sb[:, 1024 * m : 1024 * (m + 1)] for m in range(4)]
        # pass A/B chunking: two full 3584 PSUM rounds + a 1024 tail
        CH_SL = [(0, 3584), (3584, 7168), (7168, 8192)]

        def mbuf(k):        # buffer holding membrane state mh_k
            if k == 0:
                return cur
            return mA if (k % 2 == 1) else mB

        @block.sync
        def _(sync):
            sync.dma_start(out=wt_sb[:], in_=wt[:]).then_inc(dma_in, 16)
            for m in range(4):
                sync.dma_start(
                    out=xw_sb[:, 1024 * m : 1024 * (m + 1)],
                    in_=xw[:, 1024 * m : 1024 * (m + 1)],
                ).then_inc(dma_in, 16)
            sync.wait_ge(scl_g, NT // 2)
            for j in range(4):
                sync.dma_start(
                    out=g_out[2 * j : 2 * j + 2, :],
                    in_=gsb[32 * j : 32 * j + 2, :],
                ).then_inc(out_sem, 16)
            sync.wait_ge(out_sem, 64)

        @block.scalar
        def _(scalar):
            # conv: final affine per channel, trailing the DVE max
            for c in range(C):
                _, _, _, sA, sB = csc[c]
                scalar.wait_ge(cv_dve, c + 1)  # mx{c%2} written
                scalar.activation(
                    out=cur[:, 1024 * c : 1024 * (c + 1)],
                    in_=(mx0 if c % 2 == 0 else mx1)[:],
                    func=mybir.ActivationFunctionType.Copy,
                    bias=float(sB), scale=float(sA),
                ).then_inc(conv_sem)
            # g drains: the single psum bank holds steps (2k, 2k+1)
            for k in range(NT // 2):
                scalar.wait_ge(pe_g, 2 * k + 2)
                ins = None
                for j in range(4):
                    ins = scalar.copy(
                        out=gsb[32 * j : 32 * j + 2, 2 * k * B : (2 * k + 2) * B],
                        in_=g0[32 * j : 32 * j + 2, :],
                    )
                ins.then_inc(scl_g)

        @block.tensor
        def _(tensor):
            tensor.wait_ge(dma_in, 16)  # wt loaded
            # the 26 g-matmul groups (fp32, 4-way column tiling)
            for t in range(NUM_STEPS + 1):
                if t == 0:
                    tensor.wait_ge(conv_sem, C)        # mh_0 = cur ready
                elif t == NUM_STEPS:
                    tensor.wait_ge(h25, 1)             # chunks 0..27 of mh_25
                else:
                    tensor.wait_ge(dve_pi, t)          # mh_t written
                if t >= 2:
                    tensor.wait_ge(scl_g, (t - 2) // 2 + 1)  # slot drained
                col = (t % 2) * B
                mm = None
                for ch in range(NCH):
                    if t == NUM_STEPS and ch == 28:
                        tensor.wait_ge(dve_pi, NUM_STEPS)  # tail chunks ready
                    j = ch % 4
                    mm = tensor.matmul(
                        g0[32 * j : 32 * j + 2, col : col + B],
                        wt_sb[:, 2 * ch : 2 * ch + 2],
                        mbuf(t)[:, B * ch : B * (ch + 1)],
                        start=(ch < 4),
                        stop=(ch >= NCH - 4),
                        skip_group_check=True,
                        tile_position=(0, 32 * j),
                    )
                mm.then_inc(pe_g)  # pe_g = t+1

        @block.vector
        def _(vector):
            vector.wait_ge(dma_in, 80)  # all inputs resident
            # ---- conv chains per channel (all contiguous operands)
            for c in range(C):
                r01, r12, use_max, sA, sB = csc[c]
                if c >= 2:
                    vector.wait_ge(conv_sem, c - 1)  # mx{c%2} consumed by ACT
                vector.scalar_tensor_tensor(
                    out=cvE[:], in0=a_m[0][:], scalar=r01, in1=a_m[1][:],
                    op0=alu.mult, op1=alu.add,
                )
                vector.scalar_tensor_tensor(
                    out=ce2[:], in0=cvE[:], scalar=r12, in1=a_m[2][:],
                    op0=alu.mult, op1=alu.add,
                )
                vector.scalar_tensor_tensor(
                    out=cvO[:], in0=a_m[1][:], scalar=r01, in1=a_m[2][:],
                    op0=alu.mult, op1=alu.add,
                )
                vector.scalar_tensor_tensor(
                    out=co2[:], in0=cvO[:], scalar=r12, in1=a_m[3][:],
                    op0=alu.mult, op1=alu.add,
                )
                vector.tensor_tensor(
                    out=(mx0 if c % 2 == 0 else mx1)[:], in0=ce2[:], in1=co2[:],
                    op=(alu.max if use_max else alu.min),
                ).then_inc(cv_dve)  # cv_dve = c+1 : mx ready for ACT
            # ---- recurrence: per chunk, pass A u = beta*mh + CUR into PSUM,
            # pass B mh' = (mh < -1) + u reading PSUM (cheap in1 port).
            vector.wait_ge(conv_sem, C)
            for t in range(NUM_STEPS):
                vector.wait_ge(pe_g, t)  # g_{t-1} read out of mbuf(t+1)
                for ci, (lo, hi) in enumerate(CH_SL):
                    w = hi - lo
                    vector.scalar_tensor_tensor(
                        out=uP[:, 0:w], in0=mbuf(t)[:, lo:hi], scalar=BETA,
                        in1=cur[:, lo:hi], op0=alu.mult, op1=alu.add,
                    )
                    ins = vector.scalar_tensor_tensor(
                        out=mbuf(t + 1)[:, lo:hi], in0=mbuf(t)[:, lo:hi],
                        scalar=-1.0, in1=uP[:, 0:w],
                        op0=alu.is_lt, op1=alu.add,
                    )
                    if t == NUM_STEPS - 1 and ci == 1:
                        ins.then_inc(h25)   # chunks 0..27 of mh_25 ready
                ins.then_inc(dve_pi)  # dve_pi = t+1

    return nc


def _prep_inputs(x, fc_w):
    """Host-side layout prep: conv tap windows + fc weight permute."""
    x = np.ascontiguousarray(np.asarray(x, np.float32).reshape(B_FULL, L))
    x_pad = np.zeros((B_FULL, L + 3), np.float32)
    x_pad[:, 1 : L + 1] = x

    fc_w = np.asarray(fc_w, np.float32)
    # wt[p, 2*(4c+q)+o] = fc_w[o, c*4096 + 512*i + 128*q + p]
    fcv = fc_w.reshape(2, C, NCORES, 4, NP)          # [o, c, i, q, p]
    wts = []
    xws = []
    s = x_pad.strides
    for i in range(NCORES):
        arr = fcv[:, :, i]                           # [o, c, q, p]
        wt = np.ascontiguousarray(arr.transpose(3, 1, 2, 0)).reshape(NP, 2 * NCH)
        wts.append(wt)
        # xw[p, 1024m + 256q + b] = x_pad[b, 1024i + 256q + 2p + m]
        win = np.lib.stride_tricks.as_strided(
            x_pad[:, 1024 * i :],
            shape=(B_FULL, 4, NP, 4),                # [b, q, p, m]
            strides=(s[0], 256 * s[1], 2 * s[1], s[1]),
        )
        xws.append(
            np.ascontiguousarray(win.transpose(2, 3, 1, 0)).reshape(NP, 4096)
        )
    return xws, wts


def kernel(x, conv_w, conv_b, fc_w, fc_b, thr1, thr_out):
    from concourse.bass_utils import run_bass_kernel_spmd

    conv_w = np.asarray(conv_w, np.float32)
    conv_b = np.asarray(conv_b, np.float32)
    fc_b = np.asarray(fc_b, np.float64)
    thr1_f = float(np.asarray(thr1))
    thr_out_f = float(np.asarray(thr_out))

    key = (conv_w.tobytes(), conv_b.tobytes(), thr1_f)
    nc = _PROG_CACHE.get(key)
    if nc is None:
        nc = _build_nc(conv_w, conv_b, thr1_f)
        _PROG_CACHE[key] = nc

    xws, wts = _prep_inputs(x, fc_w)
    in_maps = [{"xw": xws[i], "wt": wts[i]} for i in range(NCORES)]
    res = run_bass_kernel_spmd(
        nc, in_maps, list(range(NCORES)),
        trace=PROFILE, tmpdir=TRACE_DIR,
    )
    LAST["exec_time_ns"] = res.exec_time_ns
    LAST["trace"] = res.instructions_and_trace

    # host-side: sum partial g over cores + col groups, recover cur_out, then
    # the tiny output-layer recurrence in numpy.
    gtot = np.zeros((2, NT, B), np.float64)
    for i in range(NCORES):
        g = np.asarray(res.results[i]["g_out"], np.float64)  # [8, 26*256]
        gtot += g.reshape(4, 2, NT, B).sum(axis=0)
    # g_t = -(W@m_t)/thr, so W@spk_t = g_{t+1} - beta*g_t - g_0 (thr cancels)
    wr = gtot[:, 1:] - BETA * gtot[:, :NUM_STEPS] - gtot[:, :1]
    cur_out = wr.transpose(1, 2, 0) + fc_b[None, None, :]

    mem = np.zeros((B_FULL, 2), np.float64)
    spk_rec = np.empty((NUM_STEPS, B_FULL, 2), np.float32)
    mem_rec = np.empty((NUM_STEPS, B_FULL, 2), np.float32)
    for t in range(NUM_STEPS):
        reset = (mem > thr_out_f).astype(np.float64)
        mem = BETA * mem + cur_out[t] - reset * thr_out_f
        spk_rec[t] = (mem > thr_out_f).astype(np.float32)
        mem_rec[t] = mem.astype(np.float32)
    return spk_rec, mem_rec


# revision 22
# speedup vs baseline: 1.9863x; 1.0029x over previous
"""Trainium2 Bass kernel for nn_CSNNet (conv1d -> maxpool -> 25-step LIF SNN -> fc -> LIF).

Strategy (v7): FEATURE-parallel across 8 cores.
-----------------------------------------------
Each core holds ALL 256 batches but 1/8 of the pooled feature positions
(8 channels x 512 positions = 4096 features = 32 contraction chunks of 128).
Host sums the per-core partial fc products g_t at the end.

Math: with m_t the layer-1 membrane AFTER the step-t update (m_0 = cur1), the
snntorch Leaky recurrence on the device's NEGATED NORMALIZED membrane
mh_t = -m_t/thr is
    mh_{t+1} = beta*mh_t + CUR + (mh_t < -1),   CUR = -cur1/thr = mh_0
and W@spk_t is recovered on the host from g_t = wt.T @ mh_t via
    W@spk_t ~ g_{t+1} - beta*g_t - g_0.

Engine schedule:
  DVE    : conv chains + the recurrence. Per step, pass A
           u = beta*mh + CUR writes INTO PSUM chunks, pass B
           mh' = (mh < -1) + u reads u back from PSUM: a PSUM in1 avoids the
           ~25% second-SBUF-port penalty measured on two-SBUF-source ops.
  PE     : the 26 g_t = wt.T @ mh_t accumulations (32 N=256 fp32 matmuls per
           step, 4-way column-tiled; ~4us/step, hidden under the DVE).
  ACT    : conv per-channel affine tails + PSUM->SBUF drains of g.

Conv (pad=1, k=3, maxpool2): tap-separated windows materialized host-side
(xw[p, m*1024 + q*256 + b] = x_pad[b, 1024*core + 256q + 2p + m]) make all
chain operands contiguous; per channel the DVE runs the Horner chains
e2 = (a0*r01 + a1)*r12 + a2, o2 = (a1*r01 + a2)*r12 + a3, mx = max/min, and
ACT applies CUR = mx*sA + sB.

Layout (per core)
-----------------
  partition p + chunk ch <-> channel c = ch//4, position jl = 128*(ch%4)+p
  mh/cur [128, 8192]  free index = ch*256 + b
  xw     [128, 4096]  tap m slice = [:, 1024m : 1024m+1024], inner (q, b)
  wt     [128, 64]    wt[p, 2ch+o] = fc_w[o, c*4096 + 512*core + 128*(ch%4)+p]
  uP     [128, 3584]  PSUM staging for pass A (7 banks); g: 1 bank, 2 t-slots,
                      ACT-drained every 2 steps into gsb [128, 26*256].
"""

import numpy as np

BETA = 0.9
NUM_STEPS = 25
B_FULL, L, C = 256, 8192, 8
NCORES = 8
NP = 128                        # partitions
B = B_FULL                      # batches per core (all of them)
NCH = 32                        # contraction chunks of 128 features
NT = NUM_STEPS + 1              # 26 membrane states m_0..m_25
FREE = NCH * B                  # 8192 free columns
UW = 3584                       # PSUM pass-A staging width (7 banks)

_PROG_CACHE = {}

# test-harness knobs (defaults are what the grader sees: no profiling)
PROFILE = False
TRACE_DIR = None
LAST = {}


def _conv_scalars(conv_w, conv_b, thr1):
    """Per-channel immediates for the Horner-style conv chains.

    E = w0*A(-1) + w1*A(0) + w2*A(1) + b   (even output of the pool pair)
    O = w0*A(0)  + w1*A(1) + w2*A(2) + b   (odd)
    computed as e2 = (A(-1)*(w0/w1) + A(0))*(w1/w2) + A(1)  (x w2, +b folded
    into the final affine), and max(E,O) = w2*max(e2,o2)+b for w2>0,
    w2*min(e2,o2)+b for w2<0.  Output is CUR = -(max(E,O)+b)/thr.
    """
    out = []
    for c in range(C):
        w0, w1, w2 = (float(conv_w[c, 0, d]) for d in range(3))
        b = float(conv_b[c])
        assert abs(w1) > 1e-6 and abs(w2) > 1e-6, "degenerate conv weights"
        r01 = np.float32(w0 / w1)
        r12 = np.float32(w1 / w2)
        use_max = w2 > 0
        sA = np.float32(-w2 / thr1)
        sB = np.float32(-b / thr1)
        out.append((float(r01), float(r12), use_max, float(sA), float(sB)))
    return out


def _build_nc(conv_w, conv_b, thr1):
    """Build the single-core Bass program (SPMD-identical on all 8 cores)."""
    import concourse.bass as bass
    import concourse.mybir as mybir
    from concourse.alu_op_type import AluOpType as alu
    from contextlib import ExitStack

    f32 = mybir.dt.float32
    nc = bass.Bass()
    csc = _conv_scalars(conv_w, conv_b, thr1)

    xw = nc.dram_tensor("xw", [NP, 4096], f32, kind="ExternalInput")
    wt = nc.dram_tensor("wt", [NP, 2 * NCH], f32, kind="ExternalInput")
    g_out = nc.dram_tensor("g_out", [8, NT * B], f32, kind="ExternalOutput")

    with ExitStack() as es:
        dma_in = es.enter_context(nc.semaphore("dma_in"))
        dma_m = [es.enter_context(nc.semaphore(f"dma_m{m}")) for m in range(4)]
        cv_dve = es.enter_context(nc.semaphore("cv_dve"))  # conv mx per channel
        conv_sem = es.enter_context(nc.semaphore("conv_sem"))  # ACT affine per ch
        dve_pi = es.enter_context(nc.semaphore("dve_pi"))  # dve steps done
        h25 = es.enter_context(nc.semaphore("h25"))        # chunks 0..27 of mh_25
        pe_g = es.enter_context(nc.semaphore("pe_g"))      # g-groups accumulated
        scl_g = es.enter_context(nc.semaphore("scl_g"))    # g slots drained
        out_sem = es.enter_context(nc.semaphore("out_sem"))

        xw_sb = es.enter_context(nc.sbuf_tensor("xw_sb", [NP, 4096], f32))
        wt_sb = es.enter_context(nc.sbuf_tensor("wt_sb", [NP, 2 * NCH], f32))
        cur = es.enter_context(nc.sbuf_tensor("cur", [NP, FREE], f32))
        mA = es.enter_context(nc.sbuf_tensor("mA", [NP, FREE], f32))
        mB = es.enter_context(nc.sbuf_tensor("mB", [NP, FREE], f32))
        cvE = es.enter_context(nc.sbuf_tensor("cvE", [NP, 1024], f32))
        cvO = es.enter_context(nc.sbuf_tensor("cvO", [NP, 1024], f32))
        ce2 = es.enter_context(nc.sbuf_tensor("ce2", [NP, 1024], f32))
        co2 = es.enter_context(nc.sbuf_tensor("co2", [NP, 1024], f32))
        mx0 = es.enter_context(nc.sbuf_tensor("mx0", [NP, 1024], f32))
        mx1 = es.enter_context(nc.sbuf_tensor("mx1", [NP, 1024], f32))
        gsb = es.enter_context(nc.sbuf_tensor("gsb", [NP, NT * B], f32))
        uP = es.enter_context(nc.psum_tensor("uP", [NP, UW], f32))
        g0 = es.enter_context(nc.psum_tensor("g0", [NP, 512], f32))
        block = es.enter_context(nc.Block())

        a_m = [xw_sb[:, 1024 * m : 1024 * (m + 1)] for m in range(4)]
        # pass A/B chunking: two full 3584 PSUM rounds + a 1024 tail
        CH_SL = [(0, 3584), (3584, 7168), (7168, 8192)]

        def mbuf(k):        # buffer holding membrane state mh_k
            if k == 0:
                return cur
            return mA if (k % 2 == 1) else mB

        @block.sync
        def _(sync):
            for m in range(4):
                sync.dma_start(
                    out=xw_sb[:, 1024 * m : 1024 * (m + 1)],
                    in_=xw[:, 1024 * m : 1024 * (m + 1)],
                ).then_inc(dma_m[m], 16)
            sync.dma_start(out=wt_sb[:], in_=wt[:]).then_inc(dma_in, 16)
            sync.wait_ge(scl_g, NT // 2)
            for j in range(4):
                sync.dma_start(
                    out=g_out[2 * j : 2 * j + 2, :],
                    in_=gsb[32 * j : 32 * j + 2, :],
                ).then_inc(out_sem, 16)
            sync.wait_ge(out_sem, 64)

        @block.scalar
        def _(scalar):
            # conv: final affine per channel, trailing the DVE max
            for c in range(C):
                _, _, _, sA, sB = csc[c]
                scalar.wait_ge(cv_dve, c + 1)  # mx{c%2} written
                scalar.activation(
                    out=cur[:, 1024 * c : 1024 * (c + 1)],
                    in_=(mx0 if c % 2 == 0 else mx1)[:],
                    func=mybir.ActivationFunctionType.Copy,
                    bias=float(sB), scale=float(sA),
                ).then_inc(conv_sem)
            # g drains: the single psum bank holds steps (2k, 2k+1)
            for k in range(NT // 2):
                scalar.wait_ge(pe_g, 2 * k + 2)
                ins = None
                for j in range(4):
                    ins = scalar.copy(
                        out=gsb[32 * j : 32 * j + 2, 2 * k * B : (2 * k + 2) * B],
                        in_=g0[32 * j : 32 * j + 2, :],
                    )
                ins.then_inc(scl_g)

        @block.tensor
        def _(tensor):
            tensor.wait_ge(dma_in, 16)  # wt loaded (last DMA)
            # the 26 g-matmul groups (fp32, 4-way column tiling)
            for t in range(NUM_STEPS + 1):
                if t == 0:
                    tensor.wait_ge(conv_sem, C)        # mh_0 = cur ready
                elif t == NUM_STEPS:
                    tensor.wait_ge(h25, 1)             # chunks 0..27 of mh_25
                else:
                    tensor.wait_ge(dve_pi, t)          # mh_t written
                if t >= 2:
                    tensor.wait_ge(scl_g, (t - 2) // 2 + 1)  # slot drained
                col = (t % 2) * B
                mm = None
                for ch in range(NCH):
                    if t == NUM_STEPS and ch == 28:
                        tensor.wait_ge(dve_pi, NUM_STEPS)  # tail chunks ready
                    j = ch % 4
                    mm = tensor.matmul(
                        g0[32 * j : 32 * j + 2, col : col + B],
                        wt_sb[:, 2 * ch : 2 * ch + 2],
                        mbuf(t)[:, B * ch : B * (ch + 1)],
                        start=(ch < 4),
                        stop=(ch >= NCH - 4),
                        skip_group_check=True,
                        tile_position=(0, 32 * j),
                    )
                mm.then_inc(pe_g)  # pe_g = t+1

        @block.vector
        def _(vector):
            vector.wait_ge(dma_m[0], 16)
            vector.wait_ge(dma_m[1], 16)
            # ---- conv chains per channel (all contiguous operands)
            for c in range(C):
                r01, r12, use_max, sA, sB = csc[c]
                if c >= 2:
                    vector.wait_ge(conv_sem, c - 1)  # mx{c%2} consumed by ACT
                vector.scalar_tensor_tensor(
                    out=cvE[:], in0=a_m[0][:], scalar=r01, in1=a_m[1][:],
                    op0=alu.mult, op1=alu.add,
                )
                if c == 0:
                    vector.wait_ge(dma_m[2], 16)
                vector.scalar_tensor_tensor(
                    out=ce2[:], in0=cvE[:], scalar=r12, in1=a_m[2][:],
                    op0=alu.mult, op1=alu.add,
                )
                vector.scalar_tensor_tensor(
                    out=cvO[:], in0=a_m[1][:], scalar=r01, in1=a_m[2][:],
                    op0=alu.mult, op1=alu.add,
                )
                if c == 0:
                    vector.wait_ge(dma_m[3], 16)
                vector.scalar_tensor_tensor(
                    out=co2[:], in0=cvO[:], scalar=r12, in1=a_m[3][:],
                    op0=alu.mult, op1=alu.add,
                )
                vector.tensor_tensor(
                    out=(mx0 if c % 2 == 0 else mx1)[:], in0=ce2[:], in1=co2[:],
                    op=(alu.max if use_max else alu.min),
                ).then_inc(cv_dve)  # cv_dve = c+1 : mx ready for ACT
            # ---- recurrence: per chunk, pass A u = beta*mh + CUR into PSUM,
            # pass B mh' = (mh < -1) + u reading PSUM (cheap in1 port).
            vector.wait_ge(conv_sem, C)
            for t in range(NUM_STEPS):
                vector.wait_ge(pe_g, t)  # g_{t-1} read out of mbuf(t+1)
                for ci, (lo, hi) in enumerate(CH_SL):
                    w = hi - lo
                    vector.scalar_tensor_tensor(
                        out=uP[:, 0:w], in0=mbuf(t)[:, lo:hi], scalar=BETA,
                        in1=cur[:, lo:hi], op0=alu.mult, op1=alu.add,
                    )
                    ins = vector.scalar_tensor_tensor(
                        out=mbuf(t + 1)[:, lo:hi], in0=mbuf(t)[:, lo:hi],
                        scalar=-1.0, in1=uP[:, 0:w],
                        op0=alu.is_lt, op1=alu.add,
                    )
                    if t == NUM_STEPS - 1 and ci == 1:
                        ins.then_inc(h25)   # chunks 0..27 of mh_25 ready
                ins.then_inc(dve_pi)  # dve_pi = t+1

    return nc


def _prep_inputs(x, fc_w):
    """Host-side layout prep: conv tap windows + fc weight permute."""
    x = np.ascontiguousarray(np.asarray(x, np.float32).reshape(B_FULL, L))
    x_pad = np.zeros((B_FULL, L + 3), np.float32)
    x_pad[:, 1 : L + 1] = x

    fc_w = np.asarray(fc_w, np.float32)
    # wt[p, 2*(4c+q)+o] = fc_w[o, c*4096 + 512*i + 128*q + p]
    fcv = fc_w.reshape(2, C, NCORES, 4, NP)          # [o, c, i, q, p]
    wts = []
    xws = []
    s = x_pad.strides
    for i in range(NCORES):
        arr = fcv[:, :, i]                           # [o, c, q, p]
        wt = np.ascontiguousarray(arr.transpose(3, 1, 2, 0)).reshape(NP, 2 * NCH)
        wts.append(wt)
        # xw[p, 1024m + 256q + b] = x_pad[b, 1024i + 256q + 2p + m]
        win = np.lib.stride_tricks.as_strided(
            x_pad[:, 1024 * i :],
            shape=(B_FULL, 4, NP, 4),                # [b, q, p, m]
            strides=(s[0], 256 * s[1], 2 * s[1], s[1]),
        )
        xws.append(
            np.ascontiguousarray(win.transpose(2, 3, 1, 0)).reshape(NP, 4096)
        )
    return xws, wts


def kernel(x, conv_w, conv_b, fc_w, fc_b, thr1, thr_out):
    from concourse.bass_utils import run_bass_kernel_spmd

    conv_w = np.asarray(conv_w, np.float32)
    conv_b = np.asarray(conv_b, np.float32)
    fc_b = np.asarray(fc_b, np.float64)
    thr1_f = float(np.asarray(thr1))
    thr_out_f = float(np.asarray(thr_out))

    key = (conv_w.tobytes(), conv_b.tobytes(), thr1_f)
    nc = _PROG_CACHE.get(key)
    if nc is None:
        nc = _build_nc(conv_w, conv_b, thr1_f)
        _PROG_CACHE[key] = nc

    xws, wts = _prep_inputs(x, fc_w)
    in_maps = [{"xw": xws[i], "wt": wts[i]} for i in range(NCORES)]
    res = run_bass_kernel_spmd(
        nc, in_maps, list(range(NCORES)),
        trace=PROFILE, tmpdir=TRACE_DIR,
    )
    LAST["exec_time_ns"] = res.exec_time_ns
    LAST["trace"] = res.instructions_and_trace

    # host-side: sum partial g over cores + col groups, recover cur_out, then
    # the tiny output-layer recurrence in numpy.
    gtot = np.zeros((2, NT, B), np.float64)
    for i in range(NCORES):
        g = np.asarray(res.results[i]["g_out"], np.float64)  # [8, 26*256]
        gtot += g.reshape(4, 2, NT, B).sum(axis=0)
    # g_t = -(W@m_t)/thr, so W@spk_t = g_{t+1} - beta*g_t - g_0 (thr cancels)
    wr = gtot[:, 1:] - BETA * gtot[:, :NUM_STEPS] - gtot[:, :1]
    cur_out = wr.transpose(1, 2, 0) + fc_b[None, None, :]

    mem = np.zeros((B_FULL, 2), np.float64)
    spk_rec = np.empty((NUM_STEPS, B_FULL, 2), np.float32)
    mem_rec = np.empty((NUM_STEPS, B_FULL, 2), np.float32)
    for t in range(NUM_STEPS):
        reset = (mem > thr_out_f).astype(np.float64)
        mem = BETA * mem + cur_out[t] - reset * thr_out_f
        spk_rec[t] = (mem > thr_out_f).astype(np.float32)
        mem_rec[t] = mem.astype(np.float32)
    return spk_rec, mem_rec
